# revision 37
# baseline (speedup 1.0000x reference)
"""AttentionPooling (ragged graph cross-attention pooling) on 8 TRN2 NeuronCores.

Strategy (SPMD, no collectives):
  * Host assigns 8 whole graphs to each of the 8 cores (serpentine by size),
    sorts each core's graphs by size into 8 "slots".  Slot j has a fixed tile
    count T[j] (shared by all cores, since the instruction stream is shared);
    each graph's edges are placed at its slot offset and zero-padded.
  * Host ships x^T (transposed edge features, bf16) per core + replicated
    weights.  Padding edges give exp(0)=1 in the softmax denominator, which is
    corrected with a host-computed per-slot pad count.
  * Softmax is computed without max-subtraction (scores ~ N(0,1); exp cannot
    overflow fp32) — mathematically identical to the reference's stable form.
  * Scores are linear in x: scores = (x @ w_k) . q  =  x @ Ws where
    Ws[:, (h,s)] = sum_d w_k[:, (h,d)] q[s,h,d] / sqrt(hd).
  * The whole x^T stream is preloaded into SBUF with a few large
    column-chunk DMAs (2-8KB per-partition packets) issued up-front on the
    Sync HWDGE ring, so the PE never waits on DMA mid-loop (PE idle gaps
    > 3.4us re-throttle the HAM clock gate to 1.2 GHz).  w1 is
    host-transposed to [128, 16384] and streamed on the same ring after the
    x chunks; tail-only weights go via the GPSIMD SWDGE ring so the Scalar
    engine's descriptor generator never delays the first exp.  Tiles are
    processed in PAIRS: 4 fused N=512 matmuls into one 2-bank PSUM tile
    [v|sc|v|sc], one strided exp (ACT) and one 4D-AP v-cast (DVE) per pair,
    PSUM pools bufs=3 to decouple the buffer round-trip (Tile dep tracking
    is whole-tile, so readers are emitted after all 4 matmuls).  Both
    pooled m-halves share one PSUM bank: start=True only on the m=0 first
    matmul (first_mm clears the whole bank; m=1's first matmul then
    overwrites since its has_written bits are clear).  Junk matmuls warm
    the HAM clock before the first chunk lands and across the last
    extract, so the MLP tail runs at 2.4 GHz.
  * Per graph: denom -= npad; normalize by 1/denom (DVE); 32x32 block
    transpose (DVE StreamTranspose) to build the [128, (s,half)*8graphs]
    operand P2 for the MLP (w1 needs no permutation in this layout).
  * MLP: h1 = silu(pooled @ w1 + b1) (PE, 4-way tile_position-packed), with
    sigmoid computed via the already-resident Exp table (1/(1+e^-x)) to
    avoid a ~2.7us ACT table switch; out = h1 @ w2 + b2 (PE), emitted as
    out^T [256, 8] per core; the host scatters core outputs into [64, 256].
"""

import os
import sys
from collections import deque
from contextlib import ExitStack

import numpy as np

for _p in ("/opt/trn_rl_repo",):
    if _p not in sys.path:
        sys.path.append(_p)

import ml_dtypes  # noqa: E402

import concourse.bass as bass  # noqa: E402
import concourse.tile as tile  # noqa: E402
from concourse import mybir  # noqa: E402
from concourse.bass_utils import run_bass_kernel_spmd  # noqa: E402
from concourse.vector_clock import ScopedClock  # noqa: E402

BF16 = ml_dtypes.bfloat16

E, B, H, S, NH, HD = 131072, 64, 256, 32, 8, 32
NCORES = 8
NG = B // NCORES        # graphs (slots) per core
TILE = 128              # edge tile
SCALE = 1.0 / float(np.sqrt(HD))
CHUNK0_TILES = 4        # first xt chunk (small, for fast PE start)
CHUNK_TILES = 32        # steady-state xt chunk size (tiles, even)
N_WARM_MM = 22          # junk matmuls to warm the HAM clock gate (>3.4us)
N_TAIL_WARM_MM = 22     # junk matmuls keeping the clock warm through extract

AF = mybir.ActivationFunctionType

# ---------------------------------------------------------------------------
# Walrus workaround: this toolchain's InstDrain accepts only ONE sync wait;
# Tile's kernel-tail drain carries one wait per outstanding semaphore.
# Split it into a chain of single-wait drains.
_MAXW = 1


def _split_drain_and_barrier(self, tick_clock, wait_clock):
    nc = self.nc
    drain_inst = nc.sync.drain()
    wait_clock.add_sem_waits(
        drain_inst.ins, ScopedClock({None: tick_clock.global_clock})
    )
    waits = list(drain_inst.ins.sync_info.on_wait)
    if len(waits) > _MAXW:
        drain_inst.ins.sync_info = mybir.SyncInfo(on_wait=waits[:_MAXW], on_update=[])
        for i in range(_MAXW, len(waits), _MAXW):
            d2 = nc.sync.drain()
            d2.ins.sync_info = mybir.SyncInfo(
                on_wait=waits[i : i + _MAXW], on_update=[]
            )
    nc.all_engine_barrier()
    popped = nc._tile_sem_poison_stack.pop()
    assert popped is self._sem_poison
    nc.clear_and_free_semaphores(list(self.sems.allocated().values()))
    nc.all_engine_barrier()


tile.TileContext._drain_and_barrier = _split_drain_and_barrier

# Engine instructions are capped at 2 sync waits by this walrus (Drain/NoOp
# at 1).  Tile's sem-assignment occasionally emits more.  Hoist the excess
# onto single-wait NoOps inserted just before, on the same engine — the
# engine stalls at the NoOp instead, which is semantically identical.
_WAIT_CAP = {"InstDrain": 1}
_WAIT_CAP_DEFAULT = 1


def _fix_excess_waits(nc):
    n_fixed = 0
    for fn in nc.m.functions:
        for bb in fn.blocks:
            insts = bb.instructions
            out = []
            changed = False
            for inst in insts:
                si = inst.sync_info
                waits = list(si.on_wait) if si is not None else []
                cap = _WAIT_CAP.get(type(inst).__name__, _WAIT_CAP_DEFAULT)
                if len(waits) > cap:
                    changed = True
                    n_fixed += 1
                    excess = waits[: len(waits) - cap]
                    for i, w in enumerate(excess):
                        nop = mybir.InstNoOp(
                            name=f"{inst.name}-hw{i}", ins=[], outs=[]
                        )
                        nop.engine = inst.engine
                        nop.sync_info = mybir.SyncInfo(on_wait=[w], on_update=[])
                        out.append(nop)
                    inst.sync_info = mybir.SyncInfo(
                        on_wait=waits[len(excess) :], on_update=list(si.on_update)
                    )
                out.append(inst)
            if changed:
                bb.instructions = out
    return n_fixed

# ---------------------------------------------------------------------------

_PROGRAM_CACHE: dict[tuple, "bass.Bass"] = {}
LAST_RESULTS = None  # BassKernelResults of the most recent run (for testing)


def _install_ntff_hook_shim():
    """The image's antenv lacks axon_hooks; recreate it so trace=True works."""
    try:
        import types

        import antenv

        if "antenv.axon_hooks" not in sys.modules:
            mod = types.ModuleType("antenv.axon_hooks")
            mod._hook = None

            def set_axon_ntff_profile_hook(h):
                mod._hook = h

            def get_axon_ntff_profile_hook():
                return mod._hook

            mod.set_axon_ntff_profile_hook = set_axon_ntff_profile_hook
            mod.get_axon_ntff_profile_hook = get_axon_ntff_profile_hook
            sys.modules["antenv.axon_hooks"] = mod
            antenv.axon_hooks = mod
        import antenv.axon_hooks as ah

        if ah.get_axon_ntff_profile_hook() is None:
            from trn_agent_boot.trn_boot import _ntff_profile_via_ctypes

            ah.set_axon_ntff_profile_hook(
                _ntff_profile_via_ctypes("/opt/axon/libaxon_pjrt.so")
            )
    except Exception:
        pass


_install_ntff_hook_shim()

# Shrink the kernel semaphore pool: walrus lowers the end-of-kernel
# semaphore-range reset into per-semaphore clears spread across all engines
# (~25ns each), so the default range(~54, 256) costs ~6us of epilogue.  The
# kernel only allocates ~35 semaphores (Tile pool sems + barrier/DMA lanes).
_orig_sem_range = bass.get_kernel_semaphore_range


def _small_sem_range():
    r = _orig_sem_range()
    return range(r.start, min(r.start + 72, r.stop))


bass.get_kernel_semaphore_range = _small_sem_range

# Optional experiment: let walrus double-buffer LDWEIGHTS (default off here).
import concourse.bass_utils as _bass_utils  # noqa: E402

_orig_run_command = _bass_utils.run_command


def _run_command_ldwopt(cmd, **kw):
    if isinstance(cmd, list):
        cmd = [
            "--enable-ldw-opt=true" if c == "--enable-ldw-opt=false" else c
            for c in cmd
        ]
    return _orig_run_command(cmd, **kw)


if os.environ.get("KERNEL_LDW_OPT") == "1":
    _bass_utils.run_command = _run_command_ldwopt


def _chunk_bounds(TT: int) -> list[tuple[int, int]]:
    """Even-sized tile chunks, growing: tiny first (fast PE start), then
    ramping up so the total DMA-issue count stays small (each dma_start
    costs ~0.7us of descriptor-gen on the Sync engine)."""
    sizes = [CHUNK0_TILES, 16, 32, 48]
    bounds = []
    t = 0
    i = 0
    while t < TT:
        n = min(sizes[i] if i < len(sizes) else 48, TT - t)
        i += 1
        bounds.append((t, t + n))
        t += n
    return bounds


def build_program(slot_tiles: tuple[int, ...]) -> "bass.Bass":
    """Build the SPMD Bass program for per-core slot tile counts."""
    TT = sum(slot_tiles)
    assert TT % 2 == 0
    EC = TT * TILE
    chunks = _chunk_bounds(TT)
    chunk_of = []
    for ci, (a, b) in enumerate(chunks):
        chunk_of += [ci] * (b - a)

    # per-tile slot id / first / last flags
    slot_of, first_of, last_of = [], [], []
    for j, tj in enumerate(slot_tiles):
        for t in range(tj):
            slot_of.append(j)
            first_of.append(t == 0)
            last_of.append(t == tj - 1)

    f32, bf16 = mybir.dt.float32, mybir.dt.bfloat16
    nc = bass.Bass("TRN2", target_bir_lowering=False, debug=False, num_devices=NCORES)

    xt_d = nc.dram_tensor("xt", [H, EC], bf16, kind="ExternalInput").ap()
    wvs_d = nc.dram_tensor("wvs", [H, 2 * H], bf16, kind="ExternalInput").ap()
    w1p_d = nc.dram_tensor("w1p", [128, 64 * H], bf16, kind="ExternalInput").ap()
    w2_d = nc.dram_tensor("w2", [H, H], bf16, kind="ExternalInput").ap()
    b1_d = nc.dram_tensor("b1", [NG, H], f32, kind="ExternalInput").ap()
    b2_d = nc.dram_tensor("b2", [H, 1], f32, kind="ExternalInput").ap()
    npad_d = nc.dram_tensor("npad", [128, NG], f32, kind="ExternalInput").ap()
    ident_d = nc.dram_tensor("ident", [128, 128], bf16, kind="ExternalInput").ap()
    qsel_d = nc.dram_tensor("qsel", [128, NG], bf16, kind="ExternalInput").ap()
    outT_d = nc.dram_tensor("outT", [H, NG], f32, kind="ExternalOutput").ap()

    with tile.TileContext(nc) as tc, ExitStack() as ctx:
        const = ctx.enter_context(tc.tile_pool(name="const", bufs=1))
        # k-tile k of [wv_k | ws_k]: wvs_sb[:, k*512 : k*512+256] = wv_k,
        #                            wvs_sb[:, k*512+256 : (k+1)*512] = ws_k
        wvs_sb = const.tile([128, 2 * 2 * H], bf16)
        w2_sb = const.tile([128, 2 * H], bf16)
        ident_sb = const.tile([128, 128], bf16)
        qsel_sb = const.tile([128, NG], bf16)
        b1_sb = const.tile([NG, H], f32)
        b2_sb = const.tile([128, 2], f32)
        npad_sb = const.tile([128, NG], f32)
        P2 = const.tile([128, 64 * NG], bf16)

        # Warm the ACT exp table immediately — the ONLY early scalar-engine
        # work, so the first real exp isn't stuck behind descriptor-gen.
        warm = const.tile([1, 2], f32)
        nc.gpsimd.memset(warm[:, 0:1], 0.0)
        nc.scalar.activation(warm[:, 1:2], warm[:, 0:1], AF.Exp)
        # junk operand for clock-warming matmuls (no DMA dependency)
        wsrc = const.tile([128, 512], bf16)
        nc.gpsimd.memset(wsrc[:], 0.5)

        # Everything streams on the Sync HWDGE ring (FIFO, dep-free issues
        # execute in emission order): wvs first (gates the first matmul),
        # then x chunks 0-1, the small tail weights, the remaining x chunks,
        # then w1.
        xc = [[], []]
        for ci, (a, b) in enumerate(chunks):
            for k in range(2):
                t_ = const.tile([128, (b - a) * TILE], bf16, name=f"xc{k}_{ci}")
                xc[k].append(t_)

        def dma_chunk(ci):
            a, b = chunks[ci]
            for k in range(2):
                nc.sync.dma_start(
                    xc[k][ci][:], xt_d[k * 128 : (k + 1) * 128, a * TILE : b * TILE]
                )

        dma_chunk(0)
        for k in range(2):
            r = slice(k * 128, (k + 1) * 128)
            nc.sync.dma_start(wvs_sb[:, k * 2 * H : (k + 1) * 2 * H], wvs_d[r, :])
        dma_chunk(1)
        nc.sync.dma_start(npad_sb[:], npad_d[:])
        for ci in range(2, len(chunks)):
            dma_chunk(ci)
        # tail-only weights via the GPSIMD SWDGE ring (slow per-op, but they
        # are not needed until the MLP ~80us in, and this keeps the Sync and
        # Scalar descriptor-generators free for the critical path)
        for k in range(2):
            r = slice(k * 128, (k + 1) * 128)
            nc.gpsimd.dma_start(w2_sb[:, k * H : (k + 1) * H], w2_d[r, :])
            nc.gpsimd.dma_start(b2_sb[:, k : k + 1], b2_d[r, :])
        nc.gpsimd.dma_start(ident_sb[:], ident_d[:])
        nc.gpsimd.dma_start(qsel_sb[:], qsel_d[:])
        nc.gpsimd.dma_start(b1_sb[:], b1_d[:])
        # w1 (host-pretransposed [128, 64*H]): 4 pieces, behind the x chunks
        NW1 = 4
        w1c = []
        w1w = (64 * H) // NW1
        for i in range(NW1):
            t_ = const.tile([128, w1w], bf16, name=f"w1c{i}")
            w1c.append(t_)
            nc.sync.dma_start(t_[:], w1p_d[:, i * w1w : (i + 1) * w1w])

        def w1_block(j):  # [128, H] slice for MLP k-chunk j (j = 2s+m)
            per = w1w // H
            return w1c[j // per][:, (j % per) * H : (j % per + 1) * H]

        # HAM warm-up: junk matmuls on the memset tile start right after the
        # PE preamble (no DMA dep); >=3.4us of sustained matmul flips the
        # clock gate to 2.4 GHz just before the first real matmuls arrive.
        with tc.tile_pool(name="warmp", bufs=1, space="PSUM") as wp:
            wps = wp.tile([128, 1024], f32)
            for i in range(N_WARM_MM):
                half = (i % 2) * 512
                nc.tensor.matmul(
                    wps[:, half : half + 256], wsrc[:, 0:128], wsrc[:, 0:256],
                    start=True, stop=True,
                )

        # ---- main edge loop (pairs of tiles) ----------------------------
        NRING = 6
        vs_ring = [const.tile([128, 4 * 129], bf16, name=f"vsring{i}") for i in range(NRING)]
        for t in vs_ring:
            for blk in range(4):
                nc.vector.memset(t[:, blk * 129 + 128 : blk * 129 + 129], 1.0)

        ex_pool = ctx.enter_context(tc.tile_pool(name="exp", bufs=6))
        ext_pool = ctx.enter_context(tc.tile_pool(name="ext", bufs=2))

        pooled_cur = [None]
        P2v = P2[:].rearrange("p (s x) -> p s x", x=2 * NG)

        def extract_graph(g, pl):
            # last graph is on the MLP critical path: put its m=0 copies on
            # gpsimd (overlapping the m=1 DVE chain) and m=1 copies on DVE
            for m in range(2):
                if g == NG - 1:
                    copy_eng = nc.vector if m == 1 else nc.gpsimd
                else:
                    copy_eng = nc.gpsimd
                base = m * 129
                den = ext_pool.tile([128, 1], f32, tag="den", name=f"den{g}_{m}")
                nc.vector.tensor_scalar_sub(
                    den[:], pl[:, base + 128 : base + 129], npad_sb[:, g : g + 1]
                )
                rec = ext_pool.tile([128, 1], f32, tag="rec", name=f"rec{g}_{m}")
                nc.vector.reciprocal(rec[:], den[:])
                # normalize on the Scalar engine (Copy with per-partition
                # scale) — keeps the DVE free for the v-casts / transposes
                pn = ext_pool.tile([128, 128], f32, tag="pn", name=f"pn{g}_{m}")
                nc.scalar.activation(
                    pn[:], pl[:, base : base + 128], AF.Copy, scale=rec[:]
                )
                pt = ext_pool.tile([128, 128], f32, tag="pt", name=f"pt{g}_{m}")
                nc.vector.transpose(pt[:], pn[:])
                for hh in range(4):
                    rr = slice(hh * 32, (hh + 1) * 32)
                    src = pt[rr, hh * 32 : (hh + 1) * 32].rearrange(
                        "p (a o) -> p a o", o=1
                    )
                    copy_eng.tensor_copy(P2v[rr, :, m * NG + g : m * NG + g + 1], src)

        def emit_pool(tl, tg, ex2, vs):
            sl, fi, la = slot_of[tg], first_of[tg], last_of[tg]
            if fi:
                pooled_cur[0] = pl_pool.tile(
                    [128, 258], f32, tag="pl", name=f"pl_s{sl}"
                )
            pl = pooled_cur[0]
            for m in range(2):
                # start=True only on the m=0 first matmul: first_mm clears
                # the whole bank; the m=1 region's has_written bits are then
                # unset, so its first (start=False) matmul overwrites.
                nc.tensor.matmul(
                    pl[:, m * 129 : m * 129 + 129],
                    ex2[:, tl * 256 + m * 128 : tl * 256 + m * 128 + 128],
                    vs[:, (2 * tl + m) * 129 : (2 * tl + m) * 129 + 129],
                    start=(fi and m == 0),
                    stop=la,
                    skip_group_check=True,
                )
            if la:
                extract_graph(sl, pl)

        with (
            tc.tile_pool(name="vcp", bufs=3, space="PSUM") as vc_pool,
            tc.tile_pool(name="plp", bufs=2, space="PSUM") as pl_pool,
        ):
            pending = deque()
            for p in range(TT // 2):
                t0 = 2 * p
                ci = chunk_of[t0]
                c0 = chunks[ci][0]
                # per-pair PSUM: tile t at cols t*512: [v(256) | sc(256)].
                # Tile dep-tracking is whole-tile, so exp/cast come after all
                # 4 matmuls; bufs=3 gives the round-trip enough slack.
                vsc2 = vc_pool.tile([128, 1024], f32, tag="vsc", name=f"vsc{p}")
                for tl in range(2):
                    off = (t0 + tl - c0) * TILE
                    for k in range(2):
                        nc.tensor.matmul(
                            vsc2[:, tl * 512 : (tl + 1) * 512],
                            xc[k][ci][:, off : off + TILE],
                            wvs_sb[:, k * 512 : (k + 1) * 512],
                            start=(k == 0),
                            stop=(k == 1),
                        )
                ex2 = ex_pool.tile([128, 512], bf16, tag="ex", name=f"ex{p}")
                nc.scalar.activation(
                    ex2[:].rearrange("p (t c) -> p t c", c=256),
                    vsc2[:].rearrange("p (t c) -> p t c", c=512)[:, :, 256:512],
                    AF.Exp,
                )
                vs = vs_ring[p % NRING]
                nc.vector.tensor_copy(
                    vs[:].rearrange("p (t m c) -> p t m c", t=2, c=129)[
                        :, :, :, 0:128
                    ],
                    vsc2[:].rearrange("p (t m c) -> p t m c", t=2, c=128)[
                        :, :, 0:2, :
                    ],
                )
                pending.append((p, ex2, vs))
                while len(pending) > 2:
                    q, exq, vsq = pending.popleft()
                    emit_pool(0, 2 * q, exq, vsq)
                    emit_pool(1, 2 * q + 1, exq, vsq)
            while pending:
                q, exq, vsq = pending.popleft()
                emit_pool(0, 2 * q, exq, vsq)
                emit_pool(1, 2 * q + 1, exq, vsq)

        # ---- MLP tail ----------------------------------------------------
        with (
            tc.tile_pool(name="mlpp", bufs=2, space="PSUM") as mp,
            tc.tile_pool(name="mlps", bufs=2) as ms,
        ):
            h1pp = mp.tile([128, H], f32, tag="h1pp")
            # Keep the HAM clock warm while the last slot's extract runs on
            # DVE — junk matmuls into h1pp (overwritten by the j-loop below).
            for i in range(N_TAIL_WARM_MM):
                nc.tensor.matmul(
                    h1pp[:], wvs_sb[:, 0:128], wvs_sb[:, 0:256],
                    start=True, stop=True, skip_group_check=True,
                )
            for j in range(64):
                q = j % 4
                nc.tensor.matmul(
                    h1pp[q * 32 : q * 32 + NG, :],
                    P2[:, j * NG : (j + 1) * NG],
                    w1_block(j),
                    start=(j < 4),
                    stop=(j >= 60),
                    tile_position=(0, q * 32),
                    skip_group_check=True,
                )
            # sum the 4 tile_position strips + b1 on DVE.  All strips live in
            # h1pp's single PSUM bank, and Tile serializes cross-engine reads
            # of the same bank, so a Scalar/Vector split would run SLOWER
            # than this plain 4-op DVE chain (one PSUM operand per op).
            t0 = ms.tile([NG, H], f32, tag="t0")
            nc.vector.tensor_add(t0[:], b1_sb[:], h1pp[0:NG, :])
            t1 = ms.tile([NG, H], f32, tag="t1")
            nc.vector.tensor_add(t1[:], t0[:], h1pp[32 : 32 + NG, :])
            t2 = ms.tile([NG, H], f32, tag="t2")
            nc.vector.tensor_add(t2[:], t1[:], h1pp[64 : 64 + NG, :])
            h1s = ms.tile([NG, H], bf16, tag="h1s")
            nc.vector.tensor_add(h1s[:], t2[:], h1pp[96 : 96 + NG, :])
            # transpose pre-activation, then silu on [128, NG] (full lanes)
            h1t = []
            for m in range(2):
                h1tp = mp.tile([128, NG], bf16, tag="h1tp", name=f"h1tp{m}")
                nc.tensor.transpose(
                    h1tp[:], h1s[:, m * 128 : (m + 1) * 128], ident_sb[0:NG, 0:NG]
                )
                he = ms.tile([128, NG], f32, tag="he", name=f"he{m}")
                nc.scalar.activation(he[:], h1tp[:], AF.Exp, scale=-1.0)
                ha = ms.tile([128, NG], f32, tag="ha", name=f"ha{m}")
                nc.vector.tensor_scalar_add(ha[:], he[:], 1.0)
                hr = ms.tile([128, NG], f32, tag="hr", name=f"hr{m}")
                nc.vector.reciprocal(hr[:], ha[:])
                ht = ms.tile([128, NG], bf16, tag=f"h1t{m}")
                nc.vector.tensor_mul(ht[:], h1tp[:], hr[:])
                h1t.append(ht)
            osb = ms.tile([128, 2 * NG], f32, tag="osb")
            for m in range(2):
                otp = mp.tile([128, NG], f32, tag="otp", name=f"otp{m}")
                for k in range(2):
                    nc.tensor.matmul(
                        otp[:],
                        w2_sb[:, k * H + m * 128 : k * H + m * 128 + 128],
                        h1t[k][:],
                        start=(k == 0),
                        stop=(k == 1),
                    )
                nc.vector.tensor_scalar_add(
                    osb[:, m * NG : (m + 1) * NG], otp[:], b2_sb[:, m : m + 1]
                )
            nc.sync.dma_start(
                outT_d[:].rearrange("(m p) g -> p m g", m=2),
                osb[:].rearrange("p (m g) -> p m g", m=2),
            )

    return nc


def get_program(slot_tiles: tuple[int, ...]) -> "bass.Bass":
    if slot_tiles not in _PROGRAM_CACHE:
        nc = build_program(slot_tiles)
        # HW-path only (CoreSim snapshots the program before this pass)
        _fix_excess_waits(nc)
        _PROGRAM_CACHE[slot_tiles] = nc
    return _PROGRAM_CACHE[slot_tiles]


# ---------------------------------------------------------------------------
# Host-side sharding / padding


def plan_shards(batch: np.ndarray):
    """Returns (assign [NCORES][NG] graph ids, slot_tiles tuple, sizes)."""
    sizes = np.bincount(batch, minlength=B).astype(np.int64)
    order = np.argsort(-sizes, kind="stable")
    assign = [[] for _ in range(NCORES)]
    for r in range(NG):
        row = order[r * NCORES : (r + 1) * NCORES]
        if r % 2 == 1:
            row = row[::-1]
        for c in range(NCORES):
            assign[c].append(int(row[c]))
    for c in range(NCORES):
        assign[c].sort(key=lambda g: -sizes[g])
    slot_tiles = []
    for j in range(NG):
        mx = max(sizes[assign[c][j]] for c in range(NCORES))
        slot_tiles.append(int(max(1, -(-mx // TILE))))
    # round total tiles up to an even count (pad goes to the last slot)
    rem = (-sum(slot_tiles)) % 2
    slot_tiles[-1] += rem
    return assign, tuple(slot_tiles), sizes


def make_in_maps(edge_features, batch, seed_vectors, w_q, w_k, w_v, w1, b1, w2, b2):
    edge_features = np.asarray(edge_features, dtype=np.float32)
    batch = np.asarray(batch)
    assign, slot_tiles, sizes = plan_shards(batch)
    TT = sum(slot_tiles)
    EC = TT * TILE

    starts = np.searchsorted(batch, np.arange(B))
    xb = edge_features.astype(BF16)

    # Ws[hin, h*S+s] = sum_d w_k[hin, h*HD+d] * q[s, h, d] / sqrt(HD)
    q = (np.asarray(seed_vectors, np.float32) @ np.asarray(w_q, np.float32)).reshape(
        S, NH, HD
    )
    wk3 = np.asarray(w_k, np.float32).reshape(H, NH, HD)
    Ws = (np.einsum("ihd,shd->ihs", wk3, q) * SCALE).reshape(H, NH * S)
    wvs = np.concatenate([np.asarray(w_v, np.float32), Ws], axis=1)

    # w1 pre-transposed so the device DMA is fully contiguous:
    # w1p[p, j*H + c] = w1[j*128 + p, c]
    w1p = (
        np.asarray(w1, np.float32)
        .astype(BF16)
        .reshape(64, 128, H)
        .transpose(1, 0, 2)
        .reshape(128, 64 * H)
    )

    shared = {
        "wvs": np.ascontiguousarray(wvs.astype(BF16)),
        "w1p": np.ascontiguousarray(w1p),
        "w2": np.ascontiguousarray(np.asarray(w2).astype(BF16)),
        "b1": np.ascontiguousarray(
            np.broadcast_to(np.asarray(b1, dtype=np.float32), (NG, H))
        ),
        "b2": np.ascontiguousarray(np.asarray(b2, dtype=np.float32).reshape(H, 1)),
        "ident": np.eye(128, dtype=BF16),
        "qsel": np.ascontiguousarray(
            (np.arange(128)[:, None] % 32 == np.arange(NG)[None, :]).astype(BF16)
        ),
    }

    in_maps = []
    for c in range(NCORES):
        xt = np.zeros((H, EC), dtype=BF16)
        npad = np.zeros(NG, dtype=np.float32)
        off = 0
        for j, g in enumerate(assign[c]):
            n = int(sizes[g])
            xt[:, off : off + n] = xb[starts[g] : starts[g] + n].T
            npad[j] = slot_tiles[j] * TILE - n
            off += slot_tiles[j] * TILE
        m = dict(shared)
        m["xt"] = xt
        m["npad"] = np.ascontiguousarray(np.broadcast_to(npad, (128, NG)))
        in_maps.append(m)
    return in_maps, assign, slot_tiles


def kernel(
    edge_features,
    edge_coords,
    batch,
    seed_vectors,
    w_q,
    w_k,
    w_v,
    w1,
    b1,
    w2,
    b2,
):
    in_maps, assign, slot_tiles = make_in_maps(
        edge_features, batch, seed_vectors, w_q, w_k, w_v, w1, b1, w2, b2
    )
    nc = get_program(slot_tiles)

    res = run_bass_kernel_spmd(nc, in_maps, core_ids=list(range(NCORES)))
    global LAST_RESULTS
    LAST_RESULTS = res

    out = np.zeros((B, H), dtype=np.float32)
    for c in range(NCORES):
        outT = res.results[c]["outT"]  # [H, NG]
        for j, g in enumerate(assign[c]):
            out[g, :] = outT[:, j]
    return out


# revision 38
# speedup vs baseline: 1.0371x; 1.0371x over previous
"""AttentionPooling (ragged graph cross-attention pooling) on 8 TRN2 NeuronCores.

Strategy (SPMD, no collectives):
  * Host assigns 8 whole graphs to each of the 8 cores (serpentine by size),
    sorts each core's graphs by size into 8 "slots".  Slot j has a fixed tile
    count T[j] (shared by all cores, since the instruction stream is shared);
    each graph's edges are placed at its slot offset and zero-padded.
  * Host ships x^T (transposed edge features, bf16) per core + replicated
    weights.  Padding edges give exp(0)=1 in the softmax denominator, which is
    corrected with a host-computed per-slot pad count.
  * Softmax is computed without max-subtraction (scores ~ N(0,1); exp cannot
    overflow fp32) — mathematically identical to the reference's stable form.
  * Scores are linear in x: scores = (x @ w_k) . q  =  x @ Ws where
    Ws[:, (h,s)] = sum_d w_k[:, (h,d)] q[s,h,d] / sqrt(hd).
  * The whole x^T stream is preloaded into SBUF with a few large
    column-chunk DMAs (2-8KB per-partition packets) issued up-front on the
    Sync HWDGE ring, so the PE never waits on DMA mid-loop (PE idle gaps
    > 3.4us re-throttle the HAM clock gate to 1.2 GHz).  w1 is
    host-transposed to [128, 16384] and streamed on the same ring after the
    x chunks; tail-only weights go via the GPSIMD SWDGE ring so the Scalar
    engine's descriptor generator never delays the first exp.  Tiles are
    processed in PAIRS: 4 fused N=512 matmuls into one 2-bank PSUM tile
    [v|sc|v|sc], one strided exp (ACT) and one 4D-AP v-cast (DVE) per pair,
    PSUM pools bufs=3 to decouple the buffer round-trip (Tile dep tracking
    is whole-tile, so readers are emitted after all 4 matmuls).  Both
    pooled m-halves share one PSUM bank: start=True only on the m=0 first
    matmul (first_mm clears the whole bank; m=1's first matmul then
    overwrites since its has_written bits are clear).  Junk matmuls warm
    the HAM clock before the first chunk lands and across the last
    extract, so the MLP tail runs at 2.4 GHz.
  * Per graph: denom -= npad; normalize by 1/denom (DVE); 32x32 block
    transpose (DVE StreamTranspose) to build the [128, (s,half)*8graphs]
    operand P2 for the MLP (w1 needs no permutation in this layout).
  * MLP: h1 = silu(pooled @ w1 + b1) (PE, 4-way tile_position-packed), with
    sigmoid computed via the already-resident Exp table (1/(1+e^-x)) to
    avoid a ~2.7us ACT table switch; out = h1 @ w2 + b2 (PE), emitted as
    out^T [256, 8] per core; the host scatters core outputs into [64, 256].
"""

import os
import sys
from collections import deque
from contextlib import ExitStack

import numpy as np

for _p in ("/opt/trn_rl_repo",):
    if _p not in sys.path:
        sys.path.append(_p)

import ml_dtypes  # noqa: E402

import concourse.bass as bass  # noqa: E402
import concourse.tile as tile  # noqa: E402
from concourse import mybir  # noqa: E402
from concourse.bass_utils import run_bass_kernel_spmd  # noqa: E402
from concourse.vector_clock import ScopedClock  # noqa: E402

BF16 = ml_dtypes.bfloat16

E, B, H, S, NH, HD = 131072, 64, 256, 32, 8, 32
NCORES = 8
NG = B // NCORES        # graphs (slots) per core
TILE = 128              # edge tile
SCALE = 1.0 / float(np.sqrt(HD))
CHUNK0_TILES = 4        # first xt chunk (small, for fast PE start)
CHUNK_TILES = 32        # steady-state xt chunk size (tiles, even)
N_WARM_MM = 22          # junk matmuls to warm the HAM clock gate (>3.4us)
N_TAIL_WARM_MM = 22     # junk matmuls keeping the clock warm through extract

AF = mybir.ActivationFunctionType

# ---------------------------------------------------------------------------
# Walrus workaround: this toolchain's InstDrain accepts only ONE sync wait;
# Tile's kernel-tail drain carries one wait per outstanding semaphore.
# Split it into a chain of single-wait drains.
_MAXW = 1


def _split_drain_and_barrier(self, tick_clock, wait_clock):
    nc = self.nc
    drain_inst = nc.sync.drain()
    wait_clock.add_sem_waits(
        drain_inst.ins, ScopedClock({None: tick_clock.global_clock})
    )
    waits = list(drain_inst.ins.sync_info.on_wait)
    if len(waits) > _MAXW:
        drain_inst.ins.sync_info = mybir.SyncInfo(on_wait=waits[:_MAXW], on_update=[])
        for i in range(_MAXW, len(waits), _MAXW):
            d2 = nc.sync.drain()
            d2.ins.sync_info = mybir.SyncInfo(
                on_wait=waits[i : i + _MAXW], on_update=[]
            )
    nc.all_engine_barrier()
    popped = nc._tile_sem_poison_stack.pop()
    assert popped is self._sem_poison
    nc.clear_and_free_semaphores(list(self.sems.allocated().values()))
    nc.all_engine_barrier()


tile.TileContext._drain_and_barrier = _split_drain_and_barrier

# Engine instructions are capped at 2 sync waits by this walrus (Drain/NoOp
# at 1).  Tile's sem-assignment occasionally emits more.  Hoist the excess
# onto single-wait NoOps inserted just before, on the same engine — the
# engine stalls at the NoOp instead, which is semantically identical.
_WAIT_CAP = {"InstDrain": 1}
_WAIT_CAP_DEFAULT = 1


def _fix_excess_waits(nc):
    n_fixed = 0
    for fn in nc.m.functions:
        for bb in fn.blocks:
            insts = bb.instructions
            out = []
            changed = False
            for inst in insts:
                si = inst.sync_info
                waits = list(si.on_wait) if si is not None else []
                cap = _WAIT_CAP.get(type(inst).__name__, _WAIT_CAP_DEFAULT)
                if len(waits) > cap:
                    changed = True
                    n_fixed += 1
                    excess = waits[: len(waits) - cap]
                    for i, w in enumerate(excess):
                        nop = mybir.InstNoOp(
                            name=f"{inst.name}-hw{i}", ins=[], outs=[]
                        )
                        nop.engine = inst.engine
                        nop.sync_info = mybir.SyncInfo(on_wait=[w], on_update=[])
                        out.append(nop)
                    inst.sync_info = mybir.SyncInfo(
                        on_wait=waits[len(excess) :], on_update=list(si.on_update)
                    )
                out.append(inst)
            if changed:
                bb.instructions = out
    return n_fixed

# ---------------------------------------------------------------------------

_PROGRAM_CACHE: dict[tuple, "bass.Bass"] = {}
LAST_RESULTS = None  # BassKernelResults of the most recent run (for testing)


def _install_ntff_hook_shim():
    """The image's antenv lacks axon_hooks; recreate it so trace=True works."""
    try:
        import types

        import antenv

        if "antenv.axon_hooks" not in sys.modules:
            mod = types.ModuleType("antenv.axon_hooks")
            mod._hook = None

            def set_axon_ntff_profile_hook(h):
                mod._hook = h

            def get_axon_ntff_profile_hook():
                return mod._hook

            mod.set_axon_ntff_profile_hook = set_axon_ntff_profile_hook
            mod.get_axon_ntff_profile_hook = get_axon_ntff_profile_hook
            sys.modules["antenv.axon_hooks"] = mod
            antenv.axon_hooks = mod
        import antenv.axon_hooks as ah

        if ah.get_axon_ntff_profile_hook() is None:
            from trn_agent_boot.trn_boot import _ntff_profile_via_ctypes

            ah.set_axon_ntff_profile_hook(
                _ntff_profile_via_ctypes("/opt/axon/libaxon_pjrt.so")
            )
    except Exception:
        pass


_install_ntff_hook_shim()

# Shrink the kernel semaphore pool: walrus lowers the end-of-kernel
# semaphore-range reset into per-semaphore clears spread across all engines
# (~25ns each), so the default range(~54, 256) costs ~6us of epilogue.  The
# kernel only allocates ~35 semaphores (Tile pool sems + barrier/DMA lanes).
_orig_sem_range = bass.get_kernel_semaphore_range


def _small_sem_range():
    r = _orig_sem_range()
    return range(r.start, min(r.start + 72, r.stop))


bass.get_kernel_semaphore_range = _small_sem_range

# Optional experiment: let walrus double-buffer LDWEIGHTS (default off here).
import concourse.bass_utils as _bass_utils  # noqa: E402

_orig_run_command = _bass_utils.run_command


def _run_command_ldwopt(cmd, **kw):
    if isinstance(cmd, list):
        cmd = [
            "--enable-ldw-opt=true" if c == "--enable-ldw-opt=false" else c
            for c in cmd
        ]
    return _orig_run_command(cmd, **kw)


if os.environ.get("KERNEL_LDW_OPT") == "1":
    _bass_utils.run_command = _run_command_ldwopt


def _chunk_bounds(TT: int) -> list[tuple[int, int]]:
    """Even-sized tile chunks, growing: tiny first (fast PE start), then
    ramping up so the total DMA-issue count stays small (each dma_start
    costs ~0.7us of descriptor-gen on the Sync engine)."""
    sizes = [CHUNK0_TILES, 8, 16, 32, 48]
    bounds = []
    t = 0
    i = 0
    while t < TT:
        n = min(sizes[i] if i < len(sizes) else 48, TT - t)
        i += 1
        bounds.append((t, t + n))
        t += n
    return bounds


def build_program(slot_tiles: tuple[int, ...]) -> "bass.Bass":
    """Build the SPMD Bass program for per-core slot tile counts."""
    TT = sum(slot_tiles)
    assert TT % 2 == 0
    EC = TT * TILE
    chunks = _chunk_bounds(TT)
    chunk_of = []
    for ci, (a, b) in enumerate(chunks):
        chunk_of += [ci] * (b - a)

    # per-tile slot id / first / last flags
    slot_of, first_of, last_of = [], [], []
    for j, tj in enumerate(slot_tiles):
        for t in range(tj):
            slot_of.append(j)
            first_of.append(t == 0)
            last_of.append(t == tj - 1)

    f32, bf16 = mybir.dt.float32, mybir.dt.bfloat16
    nc = bass.Bass("TRN2", target_bir_lowering=False, debug=False, num_devices=NCORES)

    xt_d = nc.dram_tensor("xt", [H, EC], bf16, kind="ExternalInput").ap()
    wvs_d = nc.dram_tensor("wvs", [H, 2 * H], bf16, kind="ExternalInput").ap()
    w1p_d = nc.dram_tensor("w1p", [128, 64 * H], bf16, kind="ExternalInput").ap()
    w2_d = nc.dram_tensor("w2", [H, H], bf16, kind="ExternalInput").ap()
    b1_d = nc.dram_tensor("b1", [NG, H], f32, kind="ExternalInput").ap()
    b2_d = nc.dram_tensor("b2", [H, 1], f32, kind="ExternalInput").ap()
    npad_d = nc.dram_tensor("npad", [128, NG], f32, kind="ExternalInput").ap()
    ident_d = nc.dram_tensor("ident", [128, 128], bf16, kind="ExternalInput").ap()
    qsel_d = nc.dram_tensor("qsel", [128, NG], bf16, kind="ExternalInput").ap()
    outT_d = nc.dram_tensor("outT", [H, NG], f32, kind="ExternalOutput").ap()

    with tile.TileContext(nc) as tc, ExitStack() as ctx:
        const = ctx.enter_context(tc.tile_pool(name="const", bufs=1))
        # k-tile k of [wv_k | ws_k]: wvs_sb[:, k*512 : k*512+256] = wv_k,
        #                            wvs_sb[:, k*512+256 : (k+1)*512] = ws_k
        wvs_sb = const.tile([128, 2 * 2 * H], bf16)
        w2_sb = const.tile([128, 2 * H], bf16)
        ident_sb = const.tile([128, 128], bf16)
        qsel_sb = const.tile([128, NG], bf16)
        b1_sb = const.tile([NG, H], f32)
        b2_sb = const.tile([128, 2], f32)
        npad_sb = const.tile([128, NG], f32)
        P2 = const.tile([128, 64 * NG], bf16)

        # Warm the ACT exp table immediately — the ONLY early scalar-engine
        # work, so the first real exp isn't stuck behind descriptor-gen.
        warm = const.tile([1, 2], f32)
        nc.gpsimd.memset(warm[:, 0:1], 0.0)
        nc.scalar.activation(warm[:, 1:2], warm[:, 0:1], AF.Exp)
        # junk operand for clock-warming matmuls (no DMA dependency)
        wsrc = const.tile([128, 512], bf16)
        nc.gpsimd.memset(wsrc[:], 0.5)

        # Everything streams on the Sync HWDGE ring (FIFO, dep-free issues
        # execute in emission order): wvs first (gates the first matmul),
        # then x chunks 0-1, the small tail weights, the remaining x chunks,
        # then w1.
        xc = [[], []]
        for ci, (a, b) in enumerate(chunks):
            for k in range(2):
                t_ = const.tile([128, (b - a) * TILE], bf16, name=f"xc{k}_{ci}")
                xc[k].append(t_)

        def dma_chunk(ci):
            a, b = chunks[ci]
            for k in range(2):
                nc.sync.dma_start(
                    xc[k][ci][:], xt_d[k * 128 : (k + 1) * 128, a * TILE : b * TILE]
                )

        dma_chunk(0)
        for k in range(2):
            r = slice(k * 128, (k + 1) * 128)
            nc.sync.dma_start(wvs_sb[:, k * 2 * H : (k + 1) * 2 * H], wvs_d[r, :])
        dma_chunk(1)
        nc.sync.dma_start(npad_sb[:], npad_d[:])
        for ci in range(2, len(chunks)):
            dma_chunk(ci)
        # tail-only weights via the GPSIMD SWDGE ring (slow per-op, but they
        # are not needed until the MLP ~80us in, and this keeps the Sync and
        # Scalar descriptor-generators free for the critical path)
        for k in range(2):
            r = slice(k * 128, (k + 1) * 128)
            nc.gpsimd.dma_start(w2_sb[:, k * H : (k + 1) * H], w2_d[r, :])
            nc.gpsimd.dma_start(b2_sb[:, k : k + 1], b2_d[r, :])
        nc.gpsimd.dma_start(ident_sb[:], ident_d[:])
        nc.gpsimd.dma_start(qsel_sb[:], qsel_d[:])
        nc.gpsimd.dma_start(b1_sb[:], b1_d[:])
        # w1 (host-pretransposed [128, 64*H]): 4 pieces, behind the x chunks
        NW1 = 4
        w1c = []
        w1w = (64 * H) // NW1
        for i in range(NW1):
            t_ = const.tile([128, w1w], bf16, name=f"w1c{i}")
            w1c.append(t_)
            nc.sync.dma_start(t_[:], w1p_d[:, i * w1w : (i + 1) * w1w])

        def w1_block(j):  # [128, H] slice for MLP k-chunk j (j = 2s+m)
            per = w1w // H
            return w1c[j // per][:, (j % per) * H : (j % per + 1) * H]

        # HAM warm-up: junk matmuls on the memset tile start right after the
        # PE preamble (no DMA dep); >=3.4us of sustained matmul flips the
        # clock gate to 2.4 GHz just before the first real matmuls arrive.
        with tc.tile_pool(name="warmp", bufs=1, space="PSUM") as wp:
            wps = wp.tile([128, 1024], f32)
            for i in range(N_WARM_MM):
                half = (i % 2) * 512
                nc.tensor.matmul(
                    wps[:, half : half + 256], wsrc[:, 0:128], wsrc[:, 0:256],
                    start=True, stop=True,
                )

        # ---- main edge loop (pairs of tiles) ----------------------------
        NRING = 6
        vs_ring = [const.tile([128, 4 * 129], bf16, name=f"vsring{i}") for i in range(NRING)]
        for t in vs_ring:
            for blk in range(4):
                nc.vector.memset(t[:, blk * 129 + 128 : blk * 129 + 129], 1.0)

        ex_pool = ctx.enter_context(tc.tile_pool(name="exp", bufs=6))
        ext_pool = ctx.enter_context(tc.tile_pool(name="ext", bufs=2))

        pooled_cur = [None]
        P2v = P2[:].rearrange("p (s x) -> p s x", x=2 * NG)

        def extract_graph(g, pl):
            # last graph is on the MLP critical path: put its m=0 copies on
            # gpsimd (overlapping the m=1 DVE chain) and m=1 copies on DVE
            for m in range(2):
                if g == NG - 1:
                    copy_eng = nc.vector if m == 1 else nc.gpsimd
                else:
                    copy_eng = nc.gpsimd
                base = m * 129
                den = ext_pool.tile([128, 1], f32, tag="den", name=f"den{g}_{m}")
                nc.vector.tensor_scalar_sub(
                    den[:], pl[:, base + 128 : base + 129], npad_sb[:, g : g + 1]
                )
                rec = ext_pool.tile([128, 1], f32, tag="rec", name=f"rec{g}_{m}")
                nc.vector.reciprocal(rec[:], den[:])
                # normalize on the Scalar engine (Copy with per-partition
                # scale) — keeps the DVE free for the v-casts / transposes
                pn = ext_pool.tile([128, 128], f32, tag="pn", name=f"pn{g}_{m}")
                nc.scalar.activation(
                    pn[:], pl[:, base : base + 128], AF.Copy, scale=rec[:]
                )
                pt = ext_pool.tile([128, 128], f32, tag="pt", name=f"pt{g}_{m}")
                nc.vector.transpose(pt[:], pn[:])
                for hh in range(4):
                    rr = slice(hh * 32, (hh + 1) * 32)
                    src = pt[rr, hh * 32 : (hh + 1) * 32].rearrange(
                        "p (a o) -> p a o", o=1
                    )
                    copy_eng.tensor_copy(P2v[rr, :, m * NG + g : m * NG + g + 1], src)

        def emit_pool(tl, tg, ex2, vs):
            sl, fi, la = slot_of[tg], first_of[tg], last_of[tg]
            if fi:
                pooled_cur[0] = pl_pool.tile(
                    [128, 258], f32, tag="pl", name=f"pl_s{sl}"
                )
            pl = pooled_cur[0]
            for m in range(2):
                # start=True only on the m=0 first matmul: first_mm clears
                # the whole bank; the m=1 region's has_written bits are then
                # unset, so its first (start=False) matmul overwrites.
                nc.tensor.matmul(
                    pl[:, m * 129 : m * 129 + 129],
                    ex2[:, tl * 256 + m * 128 : tl * 256 + m * 128 + 128],
                    vs[:, (2 * tl + m) * 129 : (2 * tl + m) * 129 + 129],
                    start=(fi and m == 0),
                    stop=la,
                    skip_group_check=True,
                )
            if la:
                extract_graph(sl, pl)

        with (
            tc.tile_pool(name="vcp", bufs=3, space="PSUM") as vc_pool,
            tc.tile_pool(name="plp", bufs=2, space="PSUM") as pl_pool,
        ):
            pending = deque()
            for p in range(TT // 2):
                t0 = 2 * p
                ci = chunk_of[t0]
                c0 = chunks[ci][0]
                # per-pair PSUM: tile t at cols t*512: [v(256) | sc(256)].
                # Tile dep-tracking is whole-tile, so exp/cast come after all
                # 4 matmuls; bufs=3 gives the round-trip enough slack.
                vsc2 = vc_pool.tile([128, 1024], f32, tag="vsc", name=f"vsc{p}")
                for tl in range(2):
                    off = (t0 + tl - c0) * TILE
                    for k in range(2):
                        nc.tensor.matmul(
                            vsc2[:, tl * 512 : (tl + 1) * 512],
                            xc[k][ci][:, off : off + TILE],
                            wvs_sb[:, k * 512 : (k + 1) * 512],
                            start=(k == 0),
                            stop=(k == 1),
                        )
                ex2 = ex_pool.tile([128, 512], bf16, tag="ex", name=f"ex{p}")
                nc.scalar.activation(
                    ex2[:].rearrange("p (t c) -> p t c", c=256),
                    vsc2[:].rearrange("p (t c) -> p t c", c=512)[:, :, 256:512],
                    AF.Exp,
                )
                vs = vs_ring[p % NRING]
                nc.vector.tensor_copy(
                    vs[:].rearrange("p (t m c) -> p t m c", t=2, c=129)[
                        :, :, :, 0:128
                    ],
                    vsc2[:].rearrange("p (t m c) -> p t m c", t=2, c=128)[
                        :, :, 0:2, :
                    ],
                )
                pending.append((p, ex2, vs))
                while len(pending) > 2:
                    q, exq, vsq = pending.popleft()
                    emit_pool(0, 2 * q, exq, vsq)
                    emit_pool(1, 2 * q + 1, exq, vsq)
            while pending:
                q, exq, vsq = pending.popleft()
                emit_pool(0, 2 * q, exq, vsq)
                emit_pool(1, 2 * q + 1, exq, vsq)

        # ---- MLP tail ----------------------------------------------------
        with (
            tc.tile_pool(name="mlpp", bufs=2, space="PSUM") as mp,
            tc.tile_pool(name="mlps", bufs=2) as ms,
        ):
            h1pp = mp.tile([128, H], f32, tag="h1pp")
            # Keep the HAM clock warm while the last slot's extract runs on
            # DVE — junk matmuls into h1pp (overwritten by the j-loop below).
            for i in range(N_TAIL_WARM_MM):
                nc.tensor.matmul(
                    h1pp[:], wvs_sb[:, 0:128], wvs_sb[:, 0:256],
                    start=True, stop=True, skip_group_check=True,
                )
            for j in range(64):
                q = j % 4
                nc.tensor.matmul(
                    h1pp[q * 32 : q * 32 + NG, :],
                    P2[:, j * NG : (j + 1) * NG],
                    w1_block(j),
                    start=(j < 4),
                    stop=(j >= 60),
                    tile_position=(0, q * 32),
                    skip_group_check=True,
                )
            # sum the 4 tile_position strips + b1 on DVE.  All strips live in
            # h1pp's single PSUM bank, and Tile serializes cross-engine reads
            # of the same bank, so a Scalar/Vector split would run SLOWER
            # than this plain 4-op DVE chain (one PSUM operand per op).
            t0 = ms.tile([NG, H], f32, tag="t0")
            nc.vector.tensor_add(t0[:], b1_sb[:], h1pp[0:NG, :])
            t1 = ms.tile([NG, H], f32, tag="t1")
            nc.vector.tensor_add(t1[:], t0[:], h1pp[32 : 32 + NG, :])
            t2 = ms.tile([NG, H], f32, tag="t2")
            nc.vector.tensor_add(t2[:], t1[:], h1pp[64 : 64 + NG, :])
            h1s = ms.tile([NG, H], bf16, tag="h1s")
            nc.vector.tensor_add(h1s[:], t2[:], h1pp[96 : 96 + NG, :])
            # transpose pre-activation, then silu on [128, NG] (full lanes)
            h1t = []
            for m in range(2):
                h1tp = mp.tile([128, NG], bf16, tag="h1tp", name=f"h1tp{m}")
                nc.tensor.transpose(
                    h1tp[:], h1s[:, m * 128 : (m + 1) * 128], ident_sb[0:NG, 0:NG]
                )
                he = ms.tile([128, NG], f32, tag="he", name=f"he{m}")
                nc.scalar.activation(he[:], h1tp[:], AF.Exp, scale=-1.0)
                ha = ms.tile([128, NG], f32, tag="ha", name=f"ha{m}")
                nc.vector.tensor_scalar_add(ha[:], he[:], 1.0)
                hr = ms.tile([128, NG], f32, tag="hr", name=f"hr{m}")
                nc.vector.reciprocal(hr[:], ha[:])
                ht = ms.tile([128, NG], bf16, tag=f"h1t{m}")
                nc.vector.tensor_mul(ht[:], h1tp[:], hr[:])
                h1t.append(ht)
            osb = ms.tile([128, 2 * NG], f32, tag="osb")
            for m in range(2):
                otp = mp.tile([128, NG], f32, tag="otp", name=f"otp{m}")
                for k in range(2):
                    nc.tensor.matmul(
                        otp[:],
                        w2_sb[:, k * H + m * 128 : k * H + m * 128 + 128],
                        h1t[k][:],
                        start=(k == 0),
                        stop=(k == 1),
                    )
                nc.vector.tensor_scalar_add(
                    osb[:, m * NG : (m + 1) * NG], otp[:], b2_sb[:, m : m + 1]
                )
            nc.sync.dma_start(
                outT_d[:].rearrange("(m p) g -> p m g", m=2),
                osb[:].rearrange("p (m g) -> p m g", m=2),
            )

    return nc


def get_program(slot_tiles: tuple[int, ...]) -> "bass.Bass":
    if slot_tiles not in _PROGRAM_CACHE:
        nc = build_program(slot_tiles)
        # HW-path only (CoreSim snapshots the program before this pass)
        _fix_excess_waits(nc)
        _PROGRAM_CACHE[slot_tiles] = nc
    return _PROGRAM_CACHE[slot_tiles]


# ---------------------------------------------------------------------------
# Host-side sharding / padding


def plan_shards(batch: np.ndarray):
    """Returns (assign [NCORES][NG] graph ids, slot_tiles tuple, sizes)."""
    sizes = np.bincount(batch, minlength=B).astype(np.int64)
    order = np.argsort(-sizes, kind="stable")
    assign = [[] for _ in range(NCORES)]
    for r in range(NG):
        row = order[r * NCORES : (r + 1) * NCORES]
        if r % 2 == 1:
            row = row[::-1]
        for c in range(NCORES):
            assign[c].append(int(row[c]))
    for c in range(NCORES):
        assign[c].sort(key=lambda g: -sizes[g])
    slot_tiles = []
    for j in range(NG):
        mx = max(sizes[assign[c][j]] for c in range(NCORES))
        slot_tiles.append(int(max(1, -(-mx // TILE))))
    # round total tiles up to an even count (pad goes to the last slot)
    rem = (-sum(slot_tiles)) % 2
    slot_tiles[-1] += rem
    return assign, tuple(slot_tiles), sizes


def make_in_maps(edge_features, batch, seed_vectors, w_q, w_k, w_v, w1, b1, w2, b2):
    edge_features = np.asarray(edge_features, dtype=np.float32)
    batch = np.asarray(batch)
    assign, slot_tiles, sizes = plan_shards(batch)
    TT = sum(slot_tiles)
    EC = TT * TILE

    starts = np.searchsorted(batch, np.arange(B))
    xb = edge_features.astype(BF16)

    # Ws[hin, h*S+s] = sum_d w_k[hin, h*HD+d] * q[s, h, d] / sqrt(HD)
    q = (np.asarray(seed_vectors, np.float32) @ np.asarray(w_q, np.float32)).reshape(
        S, NH, HD
    )
    wk3 = np.asarray(w_k, np.float32).reshape(H, NH, HD)
    Ws = (np.einsum("ihd,shd->ihs", wk3, q) * SCALE).reshape(H, NH * S)
    wvs = np.concatenate([np.asarray(w_v, np.float32), Ws], axis=1)

    # w1 pre-transposed so the device DMA is fully contiguous:
    # w1p[p, j*H + c] = w1[j*128 + p, c]
    w1p = (
        np.asarray(w1, np.float32)
        .astype(BF16)
        .reshape(64, 128, H)
        .transpose(1, 0, 2)
        .reshape(128, 64 * H)
    )

    shared = {
        "wvs": np.ascontiguousarray(wvs.astype(BF16)),
        "w1p": np.ascontiguousarray(w1p),
        "w2": np.ascontiguousarray(np.asarray(w2).astype(BF16)),
        "b1": np.ascontiguousarray(
            np.broadcast_to(np.asarray(b1, dtype=np.float32), (NG, H))
        ),
        "b2": np.ascontiguousarray(np.asarray(b2, dtype=np.float32).reshape(H, 1)),
        "ident": np.eye(128, dtype=BF16),
        "qsel": np.ascontiguousarray(
            (np.arange(128)[:, None] % 32 == np.arange(NG)[None, :]).astype(BF16)
        ),
    }

    in_maps = []
    for c in range(NCORES):
        xt = np.zeros((H, EC), dtype=BF16)
        npad = np.zeros(NG, dtype=np.float32)
        off = 0
        for j, g in enumerate(assign[c]):
            n = int(sizes[g])
            xt[:, off : off + n] = xb[starts[g] : starts[g] + n].T
            npad[j] = slot_tiles[j] * TILE - n
            off += slot_tiles[j] * TILE
        m = dict(shared)
        m["xt"] = xt
        m["npad"] = np.ascontiguousarray(np.broadcast_to(npad, (128, NG)))
        in_maps.append(m)
    return in_maps, assign, slot_tiles


def kernel(
    edge_features,
    edge_coords,
    batch,
    seed_vectors,
    w_q,
    w_k,
    w_v,
    w1,
    b1,
    w2,
    b2,
):
    in_maps, assign, slot_tiles = make_in_maps(
        edge_features, batch, seed_vectors, w_q, w_k, w_v, w1, b1, w2, b2
    )
    nc = get_program(slot_tiles)

    res = run_bass_kernel_spmd(nc, in_maps, core_ids=list(range(NCORES)))
    global LAST_RESULTS
    LAST_RESULTS = res

    out = np.zeros((B, H), dtype=np.float32)
    for c in range(NCORES):
        outT = res.results[c]["outT"]  # [H, NG]
        for j, g in enumerate(assign[c]):
            out[g, :] = outT[:, j]
    return out
